# revision 58
# baseline (speedup 1.0000x reference)
"""Causal self-attention (B=2, T=2048, C=1024, H=16) on 8 TRN2 NeuronCores.

Sharding: core c -> batch b = c//4, head-group g = c%4 (4 heads = 256 channels).
Each core computes its 4 heads end-to-end and a scaled partial projection
(1024 * y_norm_local @ W_proj[256g:+256, :] in fp16); the host rescales and
sums the 4 partials per batch.

Mixed-precision dataflow (PE fp8 DoubleRow = 0.5 cyc/col, fp16 = 1 cyc/col):
  host:  xh = e4m3(x^T), xl = e4m3((x^T - xh)*16)     (split-fp8 activations)
         Wqk = e4m3(32*W), Wvh/Wvl/Wvh16 split-fp8, Wp = fp16(32*Wp)
  qk:    q32[128ch, t] = Wqk_pair.T (x) xh_pair       (1-pass fp8 DoubleRow)
         -> bias-add-cast to fp8 pair tiles [128, 2, T] (i=1 zeroed; two
            heads packed on partition halves, d on partitions)
  v:     v32[t, ch] = xh.Wvh + xh.Wvl + xl.Wvh16      (3-pass fp8 DoubleRow)
         -> fp16 v tiles [128t, 4h*65] (65th col = 1.0 -> softmax denom)
  S^T:   per head pair-of-keytiles psum [128k, 2, 512q]: fp8 DoubleRow
         (K=64 on partitions 0:64 / 64:128 + zero half), exact causal
         windows W0 = 128*rr; bf16 identity-matmul masks (-1e30)
  P:     one exp per pair: [128, (2, W)] psum -> fp16 P tile (scale 2^-13)
  PV:    y[65, q] psum accumulated per keytile, fp16 (ones row = denom)
  norm:  recip (DVE) + partition_broadcast (Pool) + mult -> yT fp16 [128, T]
  proj:  out[t, c] = yT.T @ Wp16, fp16 staging -> fp16 partials to host

Scheduling: engines execute streams in emission(priority) order.  The
S->exp->PV chain is software-pipelined four pairs deep (S(k+4) emitted
before PV(k)) so the exp stream on ACT — the bottleneck engine at ~80us
busy — stays dense; qkv/v waves for j+1 and the proj backlog interleave
as fillers, deferred to each phase's back half where PE has slack.  The
final head's y columns [0:256) normalize + project one pair early (the
last causal window starts at 256), shortening the serial tail.
"""

import numpy as np

B, T, C = 2, 2048, 1024
H, HD = 16, 64
NCORES = 8
HEADS_PER_CORE = 4
CH = HEADS_PER_CORE * HD    # 256 channels per core
NKK = 4                     # 256-channel contraction pair-tiles for qkv
NT = T // 128               # 16 key tiles
NJ = T // 512               # 4 query chunks

_COMPILED = None


def _build():
    import concourse.bass as bass
    import concourse.bacc as bacc
    import concourse.mybir as mybir
    import concourse.tile as tile

    f32 = mybir.dt.float32
    f16 = mybir.dt.float16
    f8 = mybir.dt.float8e4
    bf16 = mybir.dt.bfloat16
    DR = mybir.MatmulPerfMode.DoubleRow
    EXP_SCALE = float(2.0 ** -13)   # 1/(sqrt(64) * 32 * 32)

    nc = bacc.Bacc("TRN2", target_bir_lowering=False, debug=False)

    xh_d = nc.dram_tensor("xh", [C, T], f8, kind="ExternalInput").ap()
    xl_d = nc.dram_tensor("xl", [C, T], f8, kind="ExternalInput").ap()
    wqk_d = nc.dram_tensor("wqk", [C, 2 * CH], f8, kind="ExternalInput").ap()
    wv3_d = nc.dram_tensor("wv3", [C, 3 * CH], f8, kind="ExternalInput").ap()
    wp_d = nc.dram_tensor("wp", [CH, C], f16, kind="ExternalInput").ap()
    msk_d = nc.dram_tensor("msk", [128, 3 * 128], bf16, kind="ExternalInput").ap()
    misc_d = nc.dram_tensor("misc", [128, 264], f32, kind="ExternalInput").ap()
    out_d = nc.dram_tensor("out_p", [T, C], f16, kind="ExternalOutput").ap()

    with tile.TileContext(nc) as tc:
        with (
            tc.tile_pool(name="p_w", bufs=1) as p_w,
            tc.tile_pool(name="p_x", bufs=1) as p_x,
            tc.tile_pool(name="p_qk", bufs=1) as p_qk,
            tc.tile_pool(name="p_v", bufs=1) as p_v,
            tc.tile_pool(name="p_y", bufs=1) as p_y,
            tc.tile_pool(name="p_p", bufs=8) as p_p,
            tc.tile_pool(name="p_sm", bufs=3) as p_sm,
            tc.tile_pool(name="p_st", bufs=6) as p_st,
            tc.tile_pool(name="ps_s", bufs=2, space="PSUM") as ps_s,
            tc.tile_pool(name="ps_y", bufs=2, space="PSUM") as ps_y,
            tc.tile_pool(name="ps_mm", bufs=2, space="PSUM") as ps_mm,
        ):
            # ---- persistent inputs (consolidated: one DMA per tensor) --
            xh_t = p_x.tile([128, NKK, 2, T], f8, name="xh", tag="xh")
            xl_t = p_x.tile([128, NKK, 2, T], f8, name="xl", tag="xl")
            wqk_t = p_w.tile([128, NKK, 2, 2 * CH], f8, name="wqk", tag="wqk")
            wv_t = p_w.tile([128, NKK, 2, 3 * CH], f8, name="wv", tag="wv")
            wp_t = p_w.tile([128, 2, C], f16, name="wp", tag="wp")
            msk_t = p_w.tile([128, 3, 128], bf16, name="msk", tag="msk")
            misc_t = p_w.tile([128, 264], f32, name="misc", tag="misc")
            bvb = p_w.tile([128, CH], f32, name="bvb", tag="bvb")
            xh = [xh_t[:, k] for k in range(NKK)]
            xl = [xl_t[:, k] for k in range(NKK)]
            wqk = [wqk_t[:, k] for k in range(NKK)]
            wvh = [wv_t[:, k, :, 0:CH] for k in range(NKK)]
            wvl = [wv_t[:, k, :, CH:2 * CH] for k in range(NKK)]
            wvh16 = [wv_t[:, k, :, 2 * CH:3 * CH] for k in range(NKK)]
            wp = [wp_t[:, k] for k in range(2)]
            tri = msk_t[:, 0]
            neg = msk_t[:, 1]
            ident = msk_t[:, 2]
            bqk = misc_t[:, 0:4]
            bvrow = misc_t[0:1, 4:4 + CH]

            # ---- persistent intermediates ------------------------------
            # q/k fp8 pair tiles: [128, 2(pair), 2(i), T]; heads (2p, 2p+1)
            # packed on partitions 0:64 / 64:128; i=1 zeroed (DoubleRow pad).
            q_all = p_qk.tile([128, 2, 2, T], f8, name="q_all", tag="q_all")
            k_all = p_qk.tile([128, 2, 2, T], f8, name="k_all", tag="k_all")
            qt = [q_all[:, p] for p in range(2)]
            kt = [k_all[:, p] for p in range(2)]
            # v tiles: [128t, 4 heads * 65] fp16 (65th col of each head = 1)
            v = [p_v.tile([128, 4 * 65], f16, name=f"v{m}", tag=f"v{m}")
                 for m in range(NT)]
            # normalized y^T fp16: tile kk holds heads (2kk, 2kk+1)
            yT = [p_y.tile([128, T], f16, name=f"yT{p}", tag=f"yT{p}")
                  for p in range(2)]

            # ---- input DMA ramp (few large DMAs: HWDGE issue is ~625ns
            # each on a shared device, so batch aggressively) -------------
            def kip(ap):
                return ap.rearrange("(k i p) c -> p k i c", k=NKK, i=2)

            # single queue, priority order (DMA_ENGINES is a serial device):
            # wave-0 inputs, masks, S zero-padding, then v-path inputs for
            # keys 0:512, then later waves
            def kip2(ap, k0):
                return ap.rearrange("(k i p) c -> p k i c", k=2, i=2)

            nc.sync.dma_start(out=wqk_t[:, 0:2], in_=kip2(wqk_d[0:512, :], 0))
            nc.sync.dma_start(out=xh_t[:, 0:2, :, 0:512],
                              in_=kip2(xh_d[0:512, 0:512], 0))
            nc.sync.dma_start(out=wqk_t[:, 2:4], in_=kip2(wqk_d[512:1024, :], 2))
            nc.sync.dma_start(out=xh_t[:, 2:4, :, 0:512],
                              in_=kip2(xh_d[512:1024, 0:512], 2))
            nc.sync.dma_start(out=msk_t,
                              in_=msk_d.rearrange("p (a c) -> p a c", a=3))
            nc.sync.dma_start(out=misc_t, in_=misc_d)
            nc.sync.dma_start(out=wv_t, in_=kip(wv3_d))
            nc.sync.dma_start(out=xl_t[:, :, :, 0:512], in_=kip(xl_d[:, 0:512]))
            nc.sync.dma_start(out=xh_t[:, :, :, 512:1024],
                              in_=kip(xh_d[:, 512:1024]))
            nc.sync.dma_start(out=xl_t[:, :, :, 512:1024],
                              in_=kip(xl_d[:, 512:1024]))
            nc.sync.dma_start(out=xh_t[:, :, :, 1024:2048],
                              in_=kip(xh_d[:, 1024:2048]))
            nc.sync.dma_start(out=xl_t[:, :, :, 1024:2048],
                              in_=kip(xl_d[:, 1024:2048]))
            nc.sync.dma_start(out=wp_t,
                              in_=wp_d.rearrange("(k p) c -> p k c", k=2))
            nc.gpsimd.memset(k_all[:, :, 1, 0:512], 0.0)
            nc.gpsimd.memset(q_all[:, :, 1, 0:512], 0.0)
            nc.gpsimd.memset(k_all[:, :, 1, 512:2048], 0.0)
            nc.gpsimd.memset(q_all[:, :, 1, 512:2048], 0.0)
            nc.gpsimd.partition_broadcast(bvb, bvrow)

            # ---- building blocks --------------------------------------
            def qk_chunk(mi, nj):
                """q or k channels [128mi, 128mi+128), t [512nj, +512).
                mi 0/1 -> q pair tiles, 2/3 -> k pair tiles."""
                blk = {0: 0, 2: 1, 1: 2, 3: 3}[mi]   # host col order q01|k01|q23|k23
                ps = ps_mm.tile([128, 512], f32, name="ps_qk", tag="mm")
                for k in range(NKK):
                    nc.tensor.matmul(
                        ps[:, 0:512],
                        lhsT=wqk[k][:, :, 128 * blk:128 * (blk + 1)],
                        rhs=xh[k][:, :, 512 * nj:512 * (nj + 1)],
                        start=(k == 0), stop=(k == NKK - 1), perf_mode=DR)
                dst = (qt[mi] if mi < 2 else kt[mi - 2])
                nc.vector.tensor_scalar_add(
                    dst[:, 0, 512 * nj:512 * (nj + 1)], ps[:, 0:512],
                    bqk[:, mi:mi + 1])

            def v_chunk(m):
                """v rows [128m, +128), all 256 channels, 3-pass split fp8."""
                ps = ps_mm.tile([128, 512], f32, name="ps_v", tag="mm")
                for k in range(NKK):
                    nc.tensor.matmul(
                        ps[:, 0:CH],
                        lhsT=xh[k][:, :, 128 * m:128 * (m + 1)],
                        rhs=wvh[k], start=(k == 0), stop=False, perf_mode=DR)
                for k in range(NKK):
                    nc.tensor.matmul(
                        ps[:, 0:CH],
                        lhsT=xh[k][:, :, 128 * m:128 * (m + 1)],
                        rhs=wvl[k], start=False, stop=False, perf_mode=DR)
                for k in range(NKK):
                    nc.tensor.matmul(
                        ps[:, 0:CH],
                        lhsT=xl[k][:, :, 128 * m:128 * (m + 1)],
                        rhs=wvh16[k], start=False, stop=(k == NKK - 1),
                        perf_mode=DR)
                for h in range(4):
                    nc.vector.memset(v[m][:, 65 * h + 64:65 * h + 65], 1.0)
                vi = v[m].rearrange("p (h c) -> p h c", h=4)[:, :, 0:64]
                nc.vector.tensor_tensor(
                    vi, ps[:, 0:CH].rearrange("p (h c) -> p h c", h=4),
                    bvb.rearrange("p (h c) -> p h c", h=4), mybir.AluOpType.add)

            yps = {}     # (j, h) -> y psum tile

            def s_exp(j, h, mp):
                """S^T + exp for head h, q-chunk j, keytile pair (2mp, 2mp+1).
                Returns (P tile, W0 of the pair window)."""
                p2, hi = divmod(h, 2)       # pair tile index, half
                qs0, qs1 = 64 * hi, 64 * (hi + 1)
                sps = ps_s.tile([128, 2, 512], f32, name="sps", tag="s")
                rr0 = 2 * mp - 4 * j        # diag offset of first tile (<0 if off-diag)
                W = [0, 0]                  # exact causal window starts
                for u in range(2):
                    i = 2 * mp + u
                    rr = i - 4 * j
                    W[u] = max(0, 128 * rr)
                    junk = rr >= 0 and W[u] > W[0]
                    if junk:
                        # the pair-exp window starts at W[0]; initialize the
                        # causally-dead prefix [W[0], W[u]) of this bank with
                        # -1e30 (opens the bank's psum group)
                        nc.tensor.matmul(
                            sps[:, u, W[0]:W[u]],
                            lhsT=ident[:, 0:128], rhs=neg[:, 0:W[u] - W[0]],
                            start=True, stop=False)
                    nc.tensor.matmul(
                        sps[:, u, W[u]:512],
                        lhsT=kt[p2][qs0:qs1, :, 128 * i:128 * (i + 1)],
                        rhs=qt[p2][qs0:qs1, :, 512 * j + W[u]:512 * (j + 1)],
                        start=not junk, stop=(rr < 0), perf_mode=DR)
                    if rr >= 0:
                        # triangular mask on the causal boundary block
                        nc.tensor.matmul(
                            sps[:, u, W[u]:W[u] + 128],
                            lhsT=ident, rhs=tri,
                            start=False, stop=True)
                W0 = W[0]
                pt = p_p.tile([128, 2, 512], f16, name="pt", tag="pt")
                nc.scalar.activation(
                    pt[:, :, W0:512], sps[:, :, W0:512],
                    mybir.ActivationFunctionType.Exp, scale=EXP_SCALE)
                return pt, W

            def norm_cols(j, h, src_y, c0, c1):
                """normalize y columns [c0, c1) of head h into yT."""
                w = c1 - c0
                rc = p_sm.tile([1, 512], f32, name="rc", tag=f"rc{h % 2}")
                bc = p_sm.tile([64, 512], f32, name="bc", tag=f"bc{h % 2}")
                nc.vector.reciprocal(rc[:, 0:w], src_y[64:65, c0:c1])
                nc.gpsimd.partition_broadcast(bc[:, 0:w], rc[0:1, 0:w])
                p2, hi = divmod(h, 2)
                nc.vector.tensor_tensor(
                    yT[p2][64 * hi:64 * (hi + 1),
                           512 * j + c0:512 * j + c1],
                    src_y[0:64, c0:c1], bc[:, 0:w], mybir.AluOpType.mult)

            def pv(j, h, mp, pt, W, last):
                """accumulate y psum for (j, h) from P pair; normalize if last."""
                tail_head = (j == NJ - 1 and h == 3)
                if mp == 0:
                    yps[(j, h)] = ps_y.tile([65, 512], f32, name="yp", tag="y")
                yp = yps[(j, h)]
                for u in range(2):
                    i = 2 * mp + u
                    nc.tensor.matmul(
                        yp[:, W[u]:512],
                        lhsT=v[i][:, 65 * h:65 * h + 65],
                        rhs=pt[:, u, W[u]:512],
                        start=(i == 0),
                        stop=(tail_head and mp == 2 * j and u == 1)
                        or (last and u == 1),
                        skip_group_check=tail_head and mp == 2 * j + 1)
                if tail_head and mp == 2 * j:
                    # cols [0:256) of the final head are complete one pair
                    # early (the last pair's causal window starts at 256):
                    # normalize + project them while the last pair finishes
                    norm_cols(j, h, yp, 0, 256)
                    proj(12, tail=True)
                    proj(13, tail=True)
                elif last:
                    if tail_head:
                        norm_cols(j, h, yp, 256, 512)
                        proj(14, tail=True)
                        proj(15, tail=True)
                    else:
                        ys = p_sm.tile([65, 512], f32, name="ys",
                                       tag=f"ys{h % 2}")
                        nc.vector.tensor_copy(ys, yp)  # frees the y psum
                        norm_cols(j, h, ys, 0, 512)

            def proj(m, tail=False):
                """output rows [128m, +128): 2 matmuls per 512-col half."""
                st = p_st.tile([128, 1024], f16, name="st", tag="st")
                for u in range(2):
                    ps = ps_mm.tile([128, 512], f32, name="ps_pr", tag="mm")
                    for kk in range(2):
                        nc.tensor.matmul(
                            ps[:, 0:512],
                            lhsT=yT[kk][:, 128 * m:128 * (m + 1)],
                            rhs=wp[kk][:, 512 * u:512 * (u + 1)],
                            start=(kk == 0), stop=(kk == 1))
                    if tail and (u + m) % 2 == 1:
                        nc.scalar.copy(st[:, 512 * u:512 * (u + 1)],
                                       ps[:, 0:512])
                    else:
                        nc.vector.tensor_copy(st[:, 512 * u:512 * (u + 1)],
                                              ps[:, 0:512])
                    if tail:
                        nc.sync.dma_start(
                            out=out_d[128 * m:128 * (m + 1),
                                      512 * u:512 * (u + 1)],
                            in_=st[:, 512 * u:512 * (u + 1)])
                if not tail:
                    nc.gpsimd.dma_start(out=out_d[128 * m:128 * (m + 1), :],
                                        in_=st)

            # ---- emission schedule ------------------------------------
            fillers = []
            slots_left = [1]
            pop_plan = []

            def fill():
                if pop_plan:
                    n = pop_plan.pop(0)
                else:
                    n = min(-(-len(fillers) // max(1, slots_left[0])), 3)
                for _ in range(n):
                    if fillers:
                        fillers.pop(0)()
                slots_left[0] -= 1

            # PE p-state warmup: ~3us of dummy matmuls on a zeroed scratch
            # tile so the first real matmuls run at full clock
            warm = p_w.tile([128, 512], f8, name="warm", tag="warm")
            nc.vector.memset(warm, 0.0)
            for w in range(18):
                wps = ps_mm.tile([128, 512], f32, name="ps_w", tag="mm")
                nc.tensor.matmul(
                    wps[:, 0:256], lhsT=warm[:, 0:256].rearrange(
                        "p (i m) -> p i m", i=2),
                    rhs=warm.rearrange("p (i m) -> p i m", i=2),
                    start=True, stop=True, perf_mode=DR)
            # wave-0 chunks for heads 0/1 only; the rest are j0 fillers
            for mi in (0, 2):
                qk_chunk(mi, 0)

            PAIRS = [(j, h, mp)
                     for j in range(NJ) for h in range(4) for mp in range(2 * j + 2)]
            pending = []
            cur_j = -1
            for (j, h, mp) in PAIRS:
                if j != cur_j:
                    while fillers:
                        fillers.pop(0)()
                    cur_j = j
                    slots_left[0] = 8 * j + 8
                    if j >= 1:       # defer fillers to the phase's back half
                        pop_plan[:] = [0] * (5 * j)
                    if j == 0:       # heads-2/3 chunks + v tiles for j=0
                        pop_plan[:] = [2, 0, 2, 2, 2]
                        for mi in (1, 3):
                            fillers.append(lambda mi=mi: qk_chunk(mi, 0))
                        for m in range(4):
                            fillers.append(lambda m=m: v_chunk(m))
                    if j + 1 < NJ:   # next wave's qkv as fillers
                        for mi in (0, 2, 1, 3):
                            fillers.append(
                                lambda mi=mi, nj=j + 1: qk_chunk(mi, nj))
                        for m in range(4 * (j + 1), 4 * (j + 2)):
                            fillers.append(lambda m=m: v_chunk(m))
                    if j == 3:       # proj backlog fills j=3's PE slack
                        for m in range(0, 12):
                            fillers.append(lambda m=m: proj(m))
                pt_w = s_exp(j, h, mp)
                pending.append((j, h, mp, pt_w[0], pt_w[1]))
                fill()
                if len(pending) > 4:   # four-pair software pipeline lag
                    pj, ph, pmp, ppt, pW = pending.pop(0)
                    pv(pj, ph, pmp, ppt, pW, last=(pmp == 2 * pj + 1))
            for (pj, ph, pmp, ppt, pW) in pending:
                pv(pj, ph, pmp, ppt, pW, last=(pmp == 2 * pj + 1))
            while fillers:
                fillers.pop(0)()

    nc.compile()
    return nc


def _host_inputs(x, W_attn, b_attn, W_proj):
    """Build the 8 per-core input maps (numpy only)."""
    import ml_dtypes
    e4 = ml_dtypes.float8_e4m3
    bf = ml_dtypes.bfloat16

    x = np.asarray(x, dtype=np.float32)
    W_attn = np.asarray(W_attn, dtype=np.float32)
    b_attn = np.asarray(b_attn, dtype=np.float32)
    W_proj = np.asarray(W_proj, dtype=np.float32)

    # causal triangular block: [k_local p, q_local c] = 0 if c >= p else -1e30
    cc = np.arange(128)[None, :]
    pp = np.arange(128)[:, None]
    tri = np.where(cc >= pp, 0.0, -1e30).astype(bf)
    neg = np.full((128, 128), -1e30, np.float32).astype(bf)
    ident = np.eye(128, dtype=bf)
    msk = np.concatenate([tri, neg, ident], axis=1)

    in_maps = []
    for c in range(NCORES):
        b, g = divmod(c, 4)
        sl = slice(CH * g, CH * (g + 1))
        xb = np.ascontiguousarray(x[b].T)                    # [C, T]
        xh = xb.astype(e4)
        xl = ((xb - xh.astype(np.float32)) * 16.0).astype(e4)
        wq = 32.0 * W_attn[:, 0 * C:1 * C][:, sl]
        wk = 32.0 * W_attn[:, 1 * C:2 * C][:, sl]
        wv = 32.0 * W_attn[:, 2 * C:3 * C][:, sl]
        wqk = np.concatenate([wq[:, 0:128], wk[:, 0:128],
                              wq[:, 128:256], wk[:, 128:256]], axis=1).astype(e4)
        wvh = wv.astype(e4)
        wvl = (wv - wvh.astype(np.float32)).astype(e4)
        wvh16 = (wvh.astype(np.float32) / 16.0).astype(e4)
        wv3 = np.concatenate([wvh, wvl, wvh16], axis=1)
        wp = (32.0 * W_proj[sl, :]).astype(np.float16)
        bq = 32.0 * b_attn[0 * C:1 * C][sl]
        bk = 32.0 * b_attn[1 * C:2 * C][sl]
        bv = 32.0 * b_attn[2 * C:3 * C][sl]
        bqk = np.stack([bq[0:128], bq[128:256], bk[0:128], bk[128:256]], axis=1)
        misc = np.zeros((128, 264), np.float32)
        misc[:, 0:4] = bqk
        misc[0, 4:4 + CH] = bv
        in_maps.append({
            "xh": xh, "xl": xl,
            "wqk": np.ascontiguousarray(wqk),
            "wv3": np.ascontiguousarray(wv3),
            "wp": np.ascontiguousarray(wp),
            "misc": misc,
            "msk": msk,
        })
    return in_maps


def kernel(x, W_attn, b_attn, W_proj, b_proj, _want_results=None):
    global _COMPILED
    from concourse.bass_utils import run_bass_kernel_spmd

    if _COMPILED is None:
        _COMPILED = _build()
    nc = _COMPILED

    in_maps = _host_inputs(x, W_attn, b_attn, W_proj)
    kw = dict(_want_results or {})
    res = run_bass_kernel_spmd(nc, in_maps, core_ids=list(range(NCORES)), **kw)
    if _want_results is not None:
        kernel.last_results = res

    out = np.zeros((B, T, C), dtype=np.float32)
    for c in range(NCORES):
        out[c // 4] += np.asarray(res.results[c]["out_p"], dtype=np.float32)
    out *= 1.0 / 1024.0
    out += np.asarray(b_proj, dtype=np.float32)[None, None, :]
    return out
